# revision 14
# baseline (speedup 1.0000x reference)
"""2-layer GCN (GCNConv x2 + relu) on 8 TRN2 NeuronCores.

Distribution: nodes dst-sharded across 8 cores (12500 each). The layer-1
dense transform (x @ W1) is computed redundantly on every core, so only
one AllGather (layer-2 transformed features) is needed.

Aggregation (per layer): messages hs[src] are gathered row-wise from an
HBM table with the Q7 dma_gather (int16 indices -> 4 src chunks of 25k
rows), then combined on the TensorEngine with a per-block selector
  S[e, v] = (dstrel[e] == v) * dinv[dst[e]]          (built on DVE)
accumulating feat-major windows in PSUM:
  agg[f, v] += sum_e G[e, f] * S[e, v]
Self-loops are read affinely (no gather): for layer 1 the node order is
ROTATED per core so its own shard sits at table rows [0, SH); for layer 2
the local t2s_shard tensor provides them. The SPMD program is identical
on all cores; all per-core variation lives in input data (indices,
rotated x, dinv columns).
"""

import os

import numpy as np
import ml_dtypes

import concourse.bacc as bacc
import concourse.mybir as mybir
from concourse.tile import TileContext
from concourse.vector_clock import VectorClock, ScopedClock
from concourse import bass_utils

BF16 = ml_dtypes.bfloat16

# ---------------------------------------------------------------------------
# TileContext drain patch: this walrus rejects >1 sync wait on a TPB_CTRL
# Drain, so split the final drain into chained single-wait drains.
# ---------------------------------------------------------------------------


def _drain_and_barrier(self, tick_clock, wait_clock):
    gc = tick_clock.global_clock
    n = len(gc)
    procs = [p for p in range(n) if gc[p] > 0]
    chunks = [procs[i : i + 1] for i in range(len(procs))] or [[]]
    for chunk in chunks:
        vc = VectorClock([gc[p] if p in chunk else 0 for p in range(n)])
        drain_inst = self.nc.sync.drain()
        wait_clock.add_sem_waits(drain_inst.ins, ScopedClock({None: vc}))
    self.nc.all_engine_barrier()
    assert self.sems is not None
    popped = self.nc._tile_sem_poison_stack.pop()
    assert popped is self._sem_poison
    self.nc.clear_and_free_semaphores(list(self.sems.allocated().values()))
    self.nc.all_engine_barrier()


TileContext._drain_and_barrier = _drain_and_barrier


# ---------------------------------------------------------------------------
# Host-side graph preprocessing
# ---------------------------------------------------------------------------


def _edge_arrays(src, dst, dinv, i, SH, CS, NCH, W, R, GRP, NGRP, rot_N):
    """Build idx_wire / dstrel / dinvd for one core and one layer.

    src: global or rotated source ids (rotation already applied by caller).
    dst: shard-local dst ids.
    Returns (idx_wire [128, cols] int16, dstrel [128, nblk] bf16,
    dinvd [128, nblk] bf16). Layout must match the static schedule:
    for g in groups: for c in chunks: for w in group: R blocks;
    then per window one self block (filled by caller).
    """
    w = dst // 128
    c = src // CS
    order = np.lexsort((src, c, w))
    s2, d2 = src[order], dst[order]
    key2 = (w * NCH + c)[order]
    starts = np.searchsorted(key2, np.arange(W * NCH))
    ends = np.searchsorted(key2, np.arange(W * NCH) + 1)

    n_gather_blocks = NGRP * NCH * 0
    gather_cols = 0
    blk = 0
    for g in range(NGRP):
        nw = min(GRP, W - g * GRP)
        blk += NCH * nw * R
        gather_cols += NCH * nw * R * 8
    n_blocks = blk + W  # + self blocks
    idx_wire = np.zeros((128, gather_cols), np.int16)
    dstrel = np.full((128, n_blocks), -1.0, np.float32)
    dinvd = np.zeros((128, n_blocks), np.float32)

    blk0 = 0
    col0 = 0
    for g in range(NGRP):
        wlo = g * GRP
        whi = min(wlo + GRP, W)
        nw = whi - wlo
        for ch in range(NCH):
            for wi in range(wlo, whi):
                k = wi * NCH + ch
                a, b = int(starts[k]), int(ends[k])
                n = b - a
                assert n <= R * 128, f"run overflow {n} > {R * 128}"
                bw0 = blk0 + (wi - wlo) * R
                j = np.arange(n)
                p = j % 128
                bb = bw0 + j // 128
                dstrel[p, bb] = d2[a:b] - wi * 128
                dinvd[p, bb] = dinv[d2[a:b] + i * SH]
                ss = (s2[a:b] - ch * CS).astype(np.int16)
                jj = (wi - wlo) * R * 128 + j
                col = col0 + jj // 16
                row = jj % 16
                for rep in range(8):
                    idx_wire[rep * 16 + row, col] = ss
            blk0 += nw * R
            col0 += nw * R * 8
    return idx_wire, dstrel, dinvd, n_blocks, gather_cols, blk0


def _preprocess(x, edge_index, W1, b1, W2, b2, n_cores=8):
    N, F = x.shape
    assert F == 128 and N % (2 * n_cores) == 0
    SH = N // n_cores
    CS = 2 * SH
    assert CS <= 32767
    NCH = N // CS
    W = (SH + 127) // 128
    assert SH % 128 == 0 or True
    GRP = int(os.environ.get("K_GRP", "4"))
    NGRP = (W + GRP - 1) // GRP

    E = edge_index.shape[1]
    src_all = np.concatenate([edge_index[0], np.arange(N, dtype=np.int64)])
    dst_all = np.concatenate([edge_index[1], np.arange(N, dtype=np.int64)])
    deg = np.bincount(dst_all, minlength=N).astype(np.float64)
    dinv = (1.0 / np.sqrt(deg)).astype(np.float32)

    # gather path handles the E real edges; appended self-loops go affine
    src_e = edge_index[0].astype(np.int64)
    dst_e = edge_index[1].astype(np.int64)

    # compute uniform R across all cores and both layers
    R = 1
    per_core_sel = []
    for i in range(n_cores):
        sel = (dst_e // SH) == i
        s = src_e[sel]
        d = dst_e[sel] - i * SH
        per_core_sel.append((s, d))
        for rot in (True, False):
            ss = (s - i * SH) % N if rot else s
            key = (d // 128) * NCH + ss // CS
            cnt = np.bincount(key, minlength=W * NCH)
            R = max(R, int((cnt.max() + 127) // 128))

    N_pad = ((N + 127) // 128) * 128
    NT = N_pad // 128
    iota = np.tile(np.arange(128, dtype=np.float32).astype(BF16), (128, 1))
    W1b = np.asarray(W1).astype(BF16)
    W2b = np.asarray(W2).astype(BF16)
    b1c = np.asarray(b1).astype(np.float32).reshape(128, 1)
    b2c = np.asarray(b2).astype(np.float32).reshape(128, 1)
    x_bf = np.asarray(x).astype(BF16)

    in_maps = []
    shape_meta = None
    for i in range(n_cores):
        s, d = per_core_sel[i]
        rs = (s - i * SH) % N
        a1 = _edge_arrays(rs, d, dinv, i, SH, CS, NCH, W, R, GRP, NGRP, N)
        a2 = _edge_arrays(s, d, dinv, i, SH, CS, NCH, W, R, GRP, NGRP, N)
        idx1, dr1, dv1, n_blocks, gather_cols, self_base = a1
        idx2, dr2, dv2, n_blocks2, gather_cols2, self_base2 = a2
        assert (n_blocks, gather_cols, self_base) == (n_blocks2, gather_cols2, self_base2)
        # self blocks (same for both layers): dstrel=iota, dinvd=dinv[own node]
        for wi in range(W):
            nb = self_base + wi
            nn = min(128, SH - wi * 128)
            p = np.arange(nn)
            for dr, dv in ((dr1, dv1), (dr2, dv2)):
                dr[p, nb] = p.astype(np.float32)
                dv[p, nb] = dinv[i * SH + wi * 128 + p]
        dw = np.zeros((128, W), np.float32)
        flat = dinv[i * SH : (i + 1) * SH]
        for wi in range(W):
            nn = min(128, SH - wi * 128)
            dw[:nn, wi] = flat[wi * 128 : wi * 128 + nn]
        # rotated inputs for the dense phase (padded to NT*128 rows)
        x_rot = np.zeros((128, N_pad), BF16)
        x_rot[:, :N] = np.roll(x_bf, -i * SH, axis=0).T
        dinv_rot = np.zeros(N_pad, np.float32)
        dinv_rot[:N] = np.roll(dinv, -i * SH)
        dcols = np.ascontiguousarray(dinv_rot.reshape(NT, 128).T)       # [128, NT]
        in_maps.append({
            "x_fm": x_rot, "W1": W1b, "W2": W2b, "iota": iota,
            "b1c": b1c, "b2c": b2c, "dinv_cols": dcols, "dinv_win": dw,
            "idx1": idx1, "dr1": dr1, "dv1": dv1,
            "idx2": idx2, "dr2": dr2, "dv2": dv2,
        })
        shape_meta = dict(
            N=N, N_pad=N_pad, SH=SH, CS=CS, NCH=NCH, W=W, GRP=GRP, NGRP=NGRP, R=R,
            n_blocks=n_blocks, gather_cols=gather_cols, self_base=self_base,
        )
    return shape_meta, in_maps


# ---------------------------------------------------------------------------
# Bass kernel builder
# ---------------------------------------------------------------------------


def _build(meta, n_cores=8):
    N = meta["N"]
    N_pad = meta["N_pad"]
    SH, CS, NCH = meta["SH"], meta["CS"], meta["NCH"]
    W, GRP, NGRP, R = meta["W"], meta["GRP"], meta["NGRP"], meta["R"]
    n_blocks = meta["n_blocks"]
    gather_cols = meta["gather_cols"]
    self_base = meta["self_base"]
    NT = N_pad // 128
    dt = mybir.dt

    nc = bacc.Bacc("TRN2", target_bir_lowering=False, debug=False)

    def inp(name, shape, dtype):
        return nc.dram_tensor(name, shape, dtype, kind="ExternalInput")

    x_fm = inp("x_fm", [128, N_pad], dt.bfloat16)
    W1 = inp("W1", [128, 128], dt.bfloat16)
    W2 = inp("W2", [128, 128], dt.bfloat16)
    iota_d = inp("iota", [128, 128], dt.bfloat16)
    b1c = inp("b1c", [128, 1], dt.float32)
    b2c = inp("b2c", [128, 1], dt.float32)
    dinv_cols = inp("dinv_cols", [128, NT], dt.float32)
    dinv_win = inp("dinv_win", [128, W], dt.float32)
    idx_d = [inp("idx1", [128, gather_cols], dt.int16),
             inp("idx2", [128, gather_cols], dt.int16)]
    dr_d = [inp("dr1", [128, n_blocks], dt.float32),
            inp("dr2", [128, n_blocks], dt.float32)]
    dv_d = [inp("dv1", [128, n_blocks], dt.float32),
            inp("dv2", [128, n_blocks], dt.float32)]

    h1s = nc.dram_tensor("h1s", [N_pad, 128], dt.bfloat16)
    t2s_shard = nc.dram_tensor("t2s_shard", [SH, 128], dt.bfloat16)
    t2s_full = nc.dram_tensor("t2s_full", [N, 128], dt.bfloat16, addr_space="Shared")
    out_d = nc.dram_tensor("out", [128, W * 128], dt.float32, kind="ExternalOutput")

    XCH = 8

    with TileContext(nc) as tc:
        with (
            tc.tile_pool(name="const", bufs=1) as constp,
            tc.tile_pool(name="idxp", bufs=1) as idxp,
            tc.tile_pool(name="selfr", bufs=1) as selfrp,
            tc.tile_pool(name="xs", bufs=3) as xs,
            tc.tile_pool(name="hstage", bufs=3) as hstage,
            tc.tile_pool(name="gbuf", bufs=2) as gbufp,
            tc.tile_pool(name="sbld", bufs=6) as sbld,
            tc.tile_pool(name="evac", bufs=4) as evacp,
            tc.tile_pool(name="t2st", bufs=3) as t2stp,
            tc.tile_pool(name="outst", bufs=3) as outstp,
            tc.tile_pool(name="psA", bufs=2, space="PSUM") as psA,
            tc.tile_pool(name="psB", bufs=2, space="PSUM") as psB,
            tc.tile_pool(name="psD", bufs=2, space="PSUM") as psD,
        ):
            w1t = constp.tile([128, 128], dt.bfloat16)
            nc.sync.dma_start(w1t[:], W1[:])
            w2t = constp.tile([128, 128], dt.bfloat16)
            nc.sync.dma_start(w2t[:], W2[:])
            iot = constp.tile([128, 128], dt.bfloat16)
            nc.sync.dma_start(iot[:], iota_d[:])
            b1t = constp.tile([128, 1], dt.float32)
            nc.sync.dma_start(b1t[:], b1c[:])
            b2t = constp.tile([128, 1], dt.float32)
            nc.sync.dma_start(b2t[:], b2c[:])
            dct = constp.tile([128, NT], dt.float32)
            nc.sync.dma_start(dct[:], dinv_cols[:])
            dwt = constp.tile([128, W], dt.float32)
            nc.sync.dma_start(dwt[:], dinv_win[:])

            idxt = idxp.tile([128, gather_cols], dt.int16, tag="idxt")
            drt = idxp.tile([128, n_blocks], dt.float32, tag="drt")
            dvt = idxp.tile([128, n_blocks], dt.float32, tag="dvt")
            # [p, w, f]: window wi's 128 own-shard rows at [:, wi, :]
            selfrows = selfrp.tile([128, W, 128], dt.bfloat16, tag="selfrows")

            # ------------- dense L1: h1s = dinv * (x @ W1) ---------------
            for tchunk in range(0, NT, XCH):
                ntile = min(XCH, NT - tchunk)
                xt = xs.tile([128, XCH * 128], dt.bfloat16, tag="xt")
                nc.sync.dma_start(
                    xt[:, : ntile * 128],
                    x_fm[:, tchunk * 128 : (tchunk + ntile) * 128],
                )
                hst = hstage.tile([128, XCH, 128], dt.bfloat16, tag="hst")
                for t in range(ntile):
                    ps = psD.tile([128, 128], dt.float32, tag="pd")
                    nc.tensor.matmul(
                        ps[:], xt[:, t * 128 : (t + 1) * 128], w1t[:],
                        start=True, stop=True,
                    )
                    nc.scalar.activation(
                        hst[:, t, :], ps[:],
                        mybir.ActivationFunctionType.Copy,
                        scale=dct[:, tchunk + t : tchunk + t + 1],
                    )
                nc.sync.dma_start(
                    h1s[tchunk * 128 : (tchunk + ntile) * 128, :].rearrange(
                        "(t p) f -> p t f", p=128
                    ),
                    hst[:, :ntile, :],
                )

            # ------------- aggregation (layer = 0 or 1) ------------------
            def agg_layer(layer, table, self_src):
                nc.sync.dma_start(idxt[:], idx_d[layer][:])
                nc.sync.dma_start(drt[:], dr_d[layer][:])
                nc.sync.dma_start(dvt[:], dv_d[layer][:])
                wfull = SH // 128
                if wfull:
                    nc.sync.dma_start(
                        selfrows[:, :wfull, :],
                        self_src[: wfull * 128, :].rearrange(
                            "(w p) f -> p w f", p=128
                        ),
                    )
                rem = SH - wfull * 128
                if rem:
                    nc.sync.dma_start(
                        selfrows[:rem, wfull, :], self_src[wfull * 128 :, :]
                    )
                blk0 = 0
                col0 = 0
                for g in range(NGRP):
                    wlo = g * GRP
                    whi = min(wlo + GRP, W)
                    nw = whi - wlo
                    nblk = nw * R
                    psg = psA.tile([128, GRP * 128], dt.float32, tag="psg")
                    gts = []
                    for ci in range(NCH):
                        gt = gbufp.tile(
                            [128, GRP * R, 128], dt.bfloat16, tag=f"gt{ci}"
                        )
                        nc.gpsimd.dma_gather(
                            gt[:, :nblk, :],
                            table[ci * CS : (ci + 1) * CS, :],
                            idxt[:, col0 + ci * nblk * 8 : col0 + (ci + 1) * nblk * 8],
                            num_idxs=nblk * 128,
                            num_idxs_reg=nblk * 128,
                            elem_size=128,
                            elem_step=128,
                            single_packet=False,
                        )
                        gts.append(gt)
                    # one contiguous PSUM accumulation chain per window
                    for wi in range(wlo, whi):
                        for ci in range(NCH):
                            bw0 = blk0 + ci * nblk + (wi - wlo) * R
                            for b in range(R):
                                gb = bw0 + b
                                st = sbld.tile([128, 128], dt.bfloat16, tag="st")
                                nc.vector.tensor_scalar(
                                    st[:], iot[:],
                                    drt[:, gb : gb + 1],
                                    dvt[:, gb : gb + 1],
                                    op0=mybir.AluOpType.is_equal,
                                    op1=mybir.AluOpType.mult,
                                )
                                nc.tensor.matmul(
                                    psg[:, (wi - wlo) * 128 : (wi - wlo + 1) * 128],
                                    gts[ci][:, (wi - wlo) * R + b, :],
                                    st[:],
                                    start=(ci == 0 and b == 0),
                                    stop=False,
                                )
                        gb = self_base + wi
                        nn = min(128, SH - wi * 128)
                        st = sbld.tile([128, 128], dt.bfloat16, tag="st")
                        nc.vector.tensor_scalar(
                            st[:], iot[:],
                            drt[:, gb : gb + 1],
                            dvt[:, gb : gb + 1],
                            op0=mybir.AluOpType.is_equal,
                            op1=mybir.AluOpType.mult,
                        )
                        nc.tensor.matmul(
                            psg[:, (wi - wlo) * 128 : (wi - wlo + 1) * 128],
                            selfrows[:nn, wi, :],
                            st[:nn, :],
                            start=False, stop=True,
                        )
                    blk0 += NCH * nblk
                    col0 += NCH * nblk * 8
                    # evacuate
                    for wi in range(wlo, whi):
                        col = (wi - wlo) * 128
                        if layer == 0:
                            hfm = evacp.tile([128, 128], dt.bfloat16, tag="hfm")
                            nc.scalar.activation(
                                hfm[:], psg[:, col : col + 128],
                                mybir.ActivationFunctionType.Identity,
                                bias=b1t[:, 0:1], scale=1.0,
                            )
                            ps2 = psB.tile([128, 128], dt.float32, tag="ps2")
                            nc.tensor.matmul(ps2[:], hfm[:], w2t[:],
                                             start=True, stop=True)
                            t2t = t2stp.tile([128, 128], dt.bfloat16, tag="t2t")
                            nc.scalar.activation(
                                t2t[:], ps2[:],
                                mybir.ActivationFunctionType.Copy,
                                scale=dwt[:, wi : wi + 1],
                            )
                            nn = min(128, SH - wi * 128)
                            nc.sync.dma_start(
                                t2s_shard[wi * 128 : wi * 128 + nn, :], t2t[:nn, :]
                            )
                        else:
                            of = outstp.tile([128, 128], dt.float32, tag="of")
                            nc.scalar.activation(
                                of[:], psg[:, col : col + 128],
                                mybir.ActivationFunctionType.Relu,
                                bias=b2t[:, 0:1], scale=1.0,
                            )
                            nc.sync.dma_start(
                                out_d[:, wi * 128 : (wi + 1) * 128], of[:]
                            )

            agg_layer(0, h1s, h1s[0:SH, :])

            nc.gpsimd.collective_compute(
                "AllGather",
                mybir.AluOpType.bypass,
                ins=[t2s_shard[:]],
                outs=[t2s_full[:]],
                replica_groups=[list(range(n_cores))],
            )

            agg_layer(1, t2s_full, t2s_shard[:])

    nc.compile()
    return nc


def kernel(x, edge_index, W1, b1, W2, b2):
    n_cores = 8
    x = np.asarray(x)
    N = x.shape[0]
    SH = N // n_cores
    meta, in_maps = _preprocess(
        x, np.asarray(edge_index), np.asarray(W1), np.asarray(b1),
        np.asarray(W2), np.asarray(b2), n_cores,
    )
    nc = _build(meta, n_cores)
    trace = bool(os.environ.get("KERNEL_TRACE"))
    res = bass_utils.run_bass_kernel_spmd(
        nc, in_maps, core_ids=list(range(n_cores)), trace=trace
    )
    global last_exec_time_ns
    last_exec_time_ns = res.exec_time_ns
    out = np.empty((N, 128), np.float32)
    for i in range(n_cores):
        o = res.results[i]["out"]
        out[i * SH : (i + 1) * SH, :] = o[:, :SH].T
    return out
